# revision 19
# baseline (speedup 1.0000x reference)
"""MoE layer (E=8, top-2, SwiGLU experts) on 8 trn2 NeuronCores.

Strategy (expert parallel, host-routed):
  - Router (flat @ router_w.T, top-2, softmax) is computed on host in fp32;
    it is tiny (33 MFLOP) and must match the reference's expert selection
    exactly (min top2-vs-3rd logit gap on these inputs is ~1e-4, far above
    fp32 matmul noise ~1e-6).
  - Tokens are dispatched to core e = expert e (the "all-to-all"), padded to
    a fixed capacity CAP. Each core runs a dense bf16 SwiGLU FFN for its
    expert over its routed tokens: yT = w2T.T @ (silu(w1T.T@xT) * (w3T.T@xT)).
    All tensors are pre-transposed AND pre-packed on host into the exact
    SBUF-resident layouts (partition-major, pack-contiguous) so every device
    DMA is a pure linear copy.
  - Host combines: out[tok] += combine_weight * y (each token appears in
    exactly 2 experts' outputs).

Compute dtype bf16 (PE runs fp32 at 1/4 rate), fp32 PSUM accumulation,
fp32 output.
"""

import os
import numpy as np
import ml_dtypes

B, S, D, H, E = 2, 2048, 1024, 2048, 8
T = B * S
TOP_K = 2
P = 128
NTOK = 512    # max token chunk (matmul free dim / one PSUM bank of fp32)
D_T = D // P  # 8 contraction slabs for stage 1 / output slabs for stage 2
H_T = H // P  # 16 hidden slabs
# w1/w3 stream in hidden-column packs: (h_start, h_count). The first packs
# are single slabs so the first gate group's critical DMA prefix is small.
PACKS13 = [(0, 1), (1, 1)] + [(2 + 2 * i, 2) for i in range(7)]
W2Q = 8       # w2 pack = 8 hidden slabs -> 2 packs

_cache = {}

# set by the last kernel() call when tracing is enabled (KERNEL_TRACE=1)
LAST_RESULTS = None


def _chunk_sizes(cap):
    """First chunk as large as possible (its stage 1 overlaps the weight
    stream, and a larger free dim slows per-h weight consumption below the
    DMA ring bandwidth); remainder split equally (multiples of 8)."""
    first = min(NTOK, cap)
    sizes = [first]
    rem = cap - first
    if rem:
        k = -(-rem // NTOK)
        base, r8 = divmod(rem // 8, k)
        sizes += [(base + (1 if i < r8 else 0)) * 8 for i in range(k)]
    chunks, s = [], 0
    for n in sizes:
        chunks.append((s, n))
        s += n
    return chunks


def _pack_x(xTe, chunks):
    """[D, cap] -> [128, D_T*cap], chunk-blocked, partition-major."""
    arr = xTe.reshape(D_T, P, -1).transpose(1, 0, 2)  # [128, D_T, cap]
    blocks = [arr[:, :, s0:s0 + n].reshape(P, D_T * n) for s0, n in chunks]
    return np.ascontiguousarray(np.concatenate(blocks, axis=1))


def _pack_w13(wT):
    """[D, H] -> [128, D_T*H], PACKS13-ordered, pack-contiguous."""
    arr = wT.reshape(D_T, P, H).transpose(1, 0, 2)            # [128, D_T, H]
    blocks = [
        arr[:, :, h0 * P:(h0 + hc) * P].reshape(P, D_T * hc * P)
        for h0, hc in PACKS13
    ]
    return np.ascontiguousarray(np.concatenate(blocks, axis=1))


def _pack_w2(w2T):
    """[H, D] -> [2, 128, W2Q*D] (per hidden-slab pack)."""
    npack = H_T // W2Q
    arr = w2T.reshape(npack, W2Q, P, D).transpose(0, 2, 1, 3)
    return np.ascontiguousarray(arr.reshape(npack, P, W2Q * D))


def _build_nc(cap, act="silu"):
    import concourse.mybir as mybir
    import concourse.tile as tile
    from concourse import bacc

    bf16 = mybir.dt.bfloat16
    f32 = mybir.dt.float32
    # "sigmoid" exists only for CoreSim smoke tests (sim lacks Silu)
    Silu = (
        mybir.ActivationFunctionType.Silu
        if act == "silu"
        else mybir.ActivationFunctionType.Sigmoid
    )

    chunks = _chunk_sizes(cap)

    nc = bacc.Bacc()
    xT_d = nc.declare_dram_parameter("xT", [P, D_T * cap], bf16, isOutput=False)
    w1T_d = nc.declare_dram_parameter("w1T", [P, D_T * H], bf16, isOutput=False)
    w3T_d = nc.declare_dram_parameter("w3T", [P, D_T * H], bf16, isOutput=False)
    w2T_d = nc.declare_dram_parameter("w2T", [H_T // W2Q, P, W2Q * D], bf16, isOutput=False)
    yT_d = nc.declare_dram_parameter("yT", [D, cap], f32, isOutput=True)

    with tile.TileContext(nc) as tc:
        with (
            tc.tile_pool(name="wpool", bufs=1) as wpool,
            tc.tile_pool(name="xpool", bufs=2) as xpool,
            tc.tile_pool(name="hpool", bufs=2) as hpool,
            tc.tile_pool(name="gpool", bufs=4) as gpool,
            tc.tile_pool(name="opool", bufs=4) as opool,
            tc.tile_pool(name="pspool", bufs=2, space="PSUM") as pspool,
        ):
            # Every load is one fully-contiguous DMA on the SP HWDGE ring, in
            # exact consumption order (x0, then w1/w3 packs interleaved, then
            # w2). Outputs use the ACT ring so they never queue behind loads.
            def load_x(ci):
                s0, n = chunks[ci]
                off = D_T * s0
                xt = xpool.tile([P, D_T, n], bf16, tag="x", name="x")
                if ci == 0:
                    # first chunk is on the critical path: halve it across rings
                    half = D_T // 2
                    nc.sync.dma_start(xt[:, :half, :], xT_d[:, off:off + half * n])
                    nc.scalar.dma_start(xt[:, half:, :], xT_d[:, off + half * n:off + D_T * n])
                else:
                    nc.sync.dma_start(xt[:], xT_d[:, off:off + D_T * n])
                return xt

            xs = load_x(0)
            # w1 packs stream on the SP ring, w3 packs on the ACT ring — the
            # two rings stay in lockstep with stage 1's (gate, up) consumption.
            # w13_tiles[h] -> (tile, column offset of slab h inside the tile)
            w1s, w3s = {}, {}
            off = 0
            for h0, hc in PACKS13:
                w = D_T * hc * P
                t1 = wpool.tile([P, D_T, hc * P], bf16, tag=f"w1_{h0}", name=f"w1_{h0}")
                nc.sync.dma_start(t1[:].rearrange("p d c -> p (d c)"), w1T_d[:, off:off + w])
                t3 = wpool.tile([P, D_T, hc * P], bf16, tag=f"w3_{h0}", name=f"w3_{h0}")
                nc.scalar.dma_start(t3[:].rearrange("p d c -> p (d c)"), w3T_d[:, off:off + w])
                for k in range(hc):
                    w1s[h0 + k] = (t1, k * P)
                    w3s[h0 + k] = (t3, k * P)
                off += w
            w2q = []
            for q in range(H_T // W2Q):
                t2 = wpool.tile([P, W2Q, D], bf16, tag=f"w2_{q}", name=f"w2_{q}")
                ring = nc.sync if q % 2 == 0 else nc.scalar
                ring.dma_start(t2[:].rearrange("p d c -> p (d c)"), w2T_d[q])
                w2q.append(t2)

            # HAM warmup: ~4us of tiny matmuls on zeros while the first loads
            # land, so the PE clock gate is already released (2.4 GHz) when
            # the real stream begins.
            warm_sb = gpool.tile([P, 16], bf16, tag="warm_sb", name="warm_sb")
            nc.vector.memset(warm_sb[:], 0.0)
            warm_ps = pspool.tile([P, 16], f32, tag="warm_ps", name="warm_ps", bufs=1)
            for _ in range(64):
                nc.tensor.matmul(
                    warm_ps[:16, :], lhsT=warm_sb[:, :16], rhs=warm_sb[:],
                    start=True, stop=True,
                )

            for ci, (s0, n) in enumerate(chunks):
                if ci > 0:
                    xs = load_x(ci)

                # stage 1: hT[h] = silu(w1T.T@xT) * (w3T.T@xT)  -> [128, n] bf16
                hts = []
                for h in range(H_T):
                    t1, c0 = w1s[h]
                    t3, c3 = w3s[h]
                    pg = pspool.tile([P, NTOK], f32, tag="pg", name="pg")
                    for d in range(D_T):
                        nc.tensor.matmul(
                            pg[:, :n],
                            lhsT=t1[:, d, c0:c0 + P],
                            rhs=xs[:, d, :],
                            start=(d == 0),
                            stop=(d == D_T - 1),
                        )
                    pu = pspool.tile([P, NTOK], f32, tag="pu", name="pu")
                    for d in range(D_T):
                        nc.tensor.matmul(
                            pu[:, :n],
                            lhsT=t3[:, d, c3:c3 + P],
                            rhs=xs[:, d, :],
                            start=(d == 0),
                            stop=(d == D_T - 1),
                        )
                    g = gpool.tile([P, NTOK], bf16, tag="g", name="g")
                    nc.scalar.activation(g[:, :n], pg[:, :n], Silu)
                    ht = hpool.tile([P, NTOK], bf16, tag=f"h_{h}", name=f"h_{h}")
                    nc.vector.tensor_mul(out=ht[:, :n], in0=g[:, :n], in1=pu[:, :n])
                    hts.append(ht)

                # stage 2: yT[do] = sum_h w2T[h,do].T @ hT[h]  -> [128, n] f32
                for do in range(D_T):
                    py = pspool.tile([P, NTOK], f32, tag="py", name="py", bufs=3)
                    for h in range(H_T):
                        nc.tensor.matmul(
                            py[:, :n],
                            lhsT=w2q[h // W2Q][:, h % W2Q, do * P:(do + 1) * P],
                            rhs=hts[h][:, :n],
                            start=(h == 0),
                            stop=(h == H_T - 1),
                        )
                    ot = opool.tile([P, NTOK], f32, tag="o", name="o")
                    nc.vector.tensor_copy(ot[:, :n], py[:, :n])
                    # output DMAs ride the ACT HWDGE ring, away from input
                    # loads; the last chunk's ride the by-then-idle SP ring
                    ring = nc.sync if ci == len(chunks) - 1 else nc.scalar
                    ring.dma_start(yT_d[do * P:(do + 1) * P, s0:s0 + n], ot[:, :n])

    nc.finalize()
    return nc


def kernel(x, router_w, w1, w2, w3):
    global LAST_RESULTS
    from concourse.bass_utils import run_bass_kernel_spmd

    x = np.ascontiguousarray(np.asarray(x, dtype=np.float32))
    router_w = np.asarray(router_w, dtype=np.float32)
    flat = x.reshape(T, D)

    # ---- host router (fp32, matches reference math) ----
    logits = flat @ router_w.T                      # [T, E]
    rows = np.arange(T)
    i1 = np.argmax(logits, axis=1)
    v1 = logits[rows, i1]
    masked = logits.copy()
    masked[rows, i1] = -np.inf
    i2 = np.argmax(masked, axis=1)
    v2 = masked[rows, i2]
    # softmax over the two selected logits (v1 >= v2)
    e2 = np.exp(v2 - v1)
    wt1 = 1.0 / (1.0 + e2)
    wt2 = e2 / (1.0 + e2)

    # ---- dispatch: token lists per expert ----
    idxs, wts = [], []
    for e in range(E):
        m1 = i1 == e
        m2 = i2 == e
        idx = np.nonzero(m1 | m2)[0]
        w = np.where(m1[idx], wt1[idx], wt2[idx]).astype(np.float32)
        idxs.append(idx)
        wts.append(w)
    max_cnt = max(len(i) for i in idxs)
    cap = max(NTOK, -(-max_cnt // 8) * 8)
    chunks = _chunk_sizes(cap)

    if cap not in _cache:
        _cache[cap] = _build_nc(cap)
    nc = _cache[cap]

    # ---- per-core inputs (bf16, pre-transposed, pre-packed) ----
    bf = ml_dtypes.bfloat16
    in_maps = []
    for e in range(E):
        idx = idxs[e]
        xTe = np.zeros((D, cap), dtype=bf)
        xTe[:, :len(idx)] = flat[idx].T.astype(bf)
        in_maps.append({
            "xT": _pack_x(xTe, chunks),
            "w1T": _pack_w13(np.ascontiguousarray(w1[e].T).astype(bf)),
            "w3T": _pack_w13(np.ascontiguousarray(w3[e].T).astype(bf)),
            "w2T": _pack_w2(np.ascontiguousarray(w2[e].T).astype(bf)),
        })

    trace = os.environ.get("KERNEL_TRACE", "0") == "1"
    kwargs = {}
    if trace:
        kwargs = dict(trace=True, trace_cores=list(range(E)))
    res = run_bass_kernel_spmd(nc, in_maps, core_ids=list(range(E)), **kwargs)
    LAST_RESULTS = res

    # ---- combine (the "all-to-all" return + weighted sum) ----
    out = np.zeros((T, D), dtype=np.float32)
    for e in range(E):
        idx = idxs[e]
        yT = res.results[e]["yT"]                   # [D, cap] f32
        out[idx] += wts[e][:, None] * yT[:, :len(idx)].T
    return out.reshape(B, S, D)


# revision 22
# speedup vs baseline: 1.2495x; 1.2495x over previous
"""MoE layer (E=8, top-2, SwiGLU experts) on 8 trn2 NeuronCores.

Strategy (expert parallel, host-routed):
  - Router (flat @ router_w.T, top-2, softmax) is computed on host in fp32;
    it is tiny (33 MFLOP) and must match the reference's expert selection
    exactly (min top2-vs-3rd logit gap on these inputs is ~1e-4, far above
    fp32 matmul noise ~1e-6).
  - Tokens are dispatched to core e = expert e (the "all-to-all"), padded to
    a fixed capacity CAP. Each core runs a dense bf16 SwiGLU FFN for its
    expert over its routed tokens: yT = w2T.T @ (silu(w1T.T@xT) * (w3T.T@xT)).
    All tensors are pre-transposed AND pre-packed on host into the exact
    SBUF-resident layouts (partition-major, pack-contiguous) so every device
    DMA is a pure linear copy.
  - Host combines: out[tok] += combine_weight * y (each token appears in
    exactly 2 experts' outputs).

Compute dtype bf16 (PE runs fp32 at 1/4 rate), fp32 PSUM accumulation,
fp32 output.
"""

import os
import numpy as np
import ml_dtypes

B, S, D, H, E = 2, 2048, 1024, 2048, 8
T = B * S
TOP_K = 2
P = 128
NTOK = 512    # max token chunk (matmul free dim / one PSUM bank of fp32)
D_T = D // P  # 8 contraction slabs for stage 1 / output slabs for stage 2
H_T = H // P  # 16 hidden slabs
# w1/w3 stream in hidden-column packs: (h_start, h_count). The first packs
# are single slabs so the first gate group's critical DMA prefix is small.
PACKS13 = [(0, 1), (1, 1)] + [(2 + 2 * i, 2) for i in range(7)]
W2Q = 8       # w2 pack = 8 hidden slabs -> 2 packs

_cache = {}

# set by the last kernel() call when tracing is enabled (KERNEL_TRACE=1)
LAST_RESULTS = None


def _chunk_sizes(cap):
    """First chunk as large as possible (its stage 1 overlaps the weight
    stream, and a larger free dim slows per-h weight consumption below the
    DMA ring bandwidth); remainder split equally (multiples of 8)."""
    first = min(NTOK, cap)
    sizes = [first]
    rem = cap - first
    if rem:
        k = -(-rem // NTOK)
        base, r8 = divmod(rem // 8, k)
        sizes += [(base + (1 if i < r8 else 0)) * 8 for i in range(k)]
    chunks, s = [], 0
    for n in sizes:
        chunks.append((s, n))
        s += n
    return chunks


def _pack_x(xTe, chunks):
    """[D, cap] -> [128, D_T*cap], chunk-blocked, partition-major."""
    arr = xTe.reshape(D_T, P, -1).transpose(1, 0, 2)  # [128, D_T, cap]
    blocks = [arr[:, :, s0:s0 + n].reshape(P, D_T * n) for s0, n in chunks]
    return np.ascontiguousarray(np.concatenate(blocks, axis=1))


def _pack_w13(wT):
    """[D, H] -> [128, D_T*H], PACKS13-ordered, pack-contiguous."""
    arr = wT.reshape(D_T, P, H).transpose(1, 0, 2)            # [128, D_T, H]
    blocks = [
        arr[:, :, h0 * P:(h0 + hc) * P].reshape(P, D_T * hc * P)
        for h0, hc in PACKS13
    ]
    return np.ascontiguousarray(np.concatenate(blocks, axis=1))


def _pack_w2(w2T):
    """[H, D] -> [2, 128, W2Q*D] (per hidden-slab pack)."""
    npack = H_T // W2Q
    arr = w2T.reshape(npack, W2Q, P, D).transpose(0, 2, 1, 3)
    return np.ascontiguousarray(arr.reshape(npack, P, W2Q * D))


def _build_nc(cap, act="silu"):
    import concourse.mybir as mybir
    import concourse.tile as tile
    from concourse import bacc

    bf16 = mybir.dt.bfloat16
    f32 = mybir.dt.float32
    # "sigmoid" exists only for CoreSim smoke tests (sim lacks Silu)
    Silu = (
        mybir.ActivationFunctionType.Silu
        if act == "silu"
        else mybir.ActivationFunctionType.Sigmoid
    )

    chunks = _chunk_sizes(cap)

    nc = bacc.Bacc()
    xT_d = nc.declare_dram_parameter("xT", [P, D_T * cap], bf16, isOutput=False)
    w1T_d = nc.declare_dram_parameter("w1T", [P, D_T * H], bf16, isOutput=False)
    w3T_d = nc.declare_dram_parameter("w3T", [P, D_T * H], bf16, isOutput=False)
    w2T_d = nc.declare_dram_parameter("w2T", [H_T // W2Q, P, W2Q * D], bf16, isOutput=False)
    yT_d = nc.declare_dram_parameter("yT", [D, cap], f32, isOutput=True)

    with tile.TileContext(nc) as tc:
        with (
            tc.tile_pool(name="wpool", bufs=1) as wpool,
            tc.tile_pool(name="xpool", bufs=2) as xpool,
            tc.tile_pool(name="hpool", bufs=2) as hpool,
            tc.tile_pool(name="gpool", bufs=4) as gpool,
            tc.tile_pool(name="opool", bufs=4) as opool,
            tc.tile_pool(name="pspool", bufs=2, space="PSUM") as pspool,
        ):
            # Every load is one fully-contiguous DMA on the SP HWDGE ring, in
            # exact consumption order (x0, then w1/w3 packs interleaved, then
            # w2). Outputs use the ACT ring so they never queue behind loads.
            def load_x(ci):
                s0, n = chunks[ci]
                off = D_T * s0
                xt = xpool.tile([P, D_T, n], bf16, tag="x", name="x")
                nc.sync.dma_start(xt[:], xT_d[:, off:off + D_T * n])
                return xt

            xs = load_x(0)
            # All input loads ride the SP HWDGE ring in consumption order.
            # (Putting loads on the ACT ring measurably hurts: its DMAs share
            # the Scalar sequencer with the Silu activations.)
            # w13_tiles[h] -> (tile, column offset of slab h inside the tile)
            w1s, w3s = {}, {}
            off = 0
            for h0, hc in PACKS13:
                w = D_T * hc * P
                t1 = wpool.tile([P, D_T, hc * P], bf16, tag=f"w1_{h0}", name=f"w1_{h0}")
                nc.sync.dma_start(t1[:].rearrange("p d c -> p (d c)"), w1T_d[:, off:off + w])
                t3 = wpool.tile([P, D_T, hc * P], bf16, tag=f"w3_{h0}", name=f"w3_{h0}")
                nc.sync.dma_start(t3[:].rearrange("p d c -> p (d c)"), w3T_d[:, off:off + w])
                for k in range(hc):
                    w1s[h0 + k] = (t1, k * P)
                    w3s[h0 + k] = (t3, k * P)
                off += w
            w2q = []
            for q in range(H_T // W2Q):
                t2 = wpool.tile([P, W2Q, D], bf16, tag=f"w2_{q}", name=f"w2_{q}")
                nc.sync.dma_start(t2[:].rearrange("p d c -> p (d c)"), w2T_d[q])
                w2q.append(t2)

            # HAM warmup: ~4us of tiny matmuls on zeros while the first loads
            # land, so the PE clock gate is already released (2.4 GHz) when
            # the real stream begins.
            warm_sb = gpool.tile([P, 16], bf16, tag="warm_sb", name="warm_sb")
            nc.vector.memset(warm_sb[:], 0.0)
            warm_ps = pspool.tile([P, 16], f32, tag="warm_ps", name="warm_ps", bufs=1)
            for _ in range(64):
                nc.tensor.matmul(
                    warm_ps[:16, :], lhsT=warm_sb[:, :16], rhs=warm_sb[:],
                    start=True, stop=True,
                )

            for ci, (s0, n) in enumerate(chunks):
                if ci > 0:
                    xs = load_x(ci)

                # stage 1: hT[h] = silu(w1T.T@xT) * (w3T.T@xT)  -> [128, n] bf16
                hts = []
                for h in range(H_T):
                    t1, c0 = w1s[h]
                    t3, c3 = w3s[h]
                    pg = pspool.tile([P, NTOK], f32, tag="pg", name="pg")
                    for d in range(D_T):
                        nc.tensor.matmul(
                            pg[:, :n],
                            lhsT=t1[:, d, c0:c0 + P],
                            rhs=xs[:, d, :],
                            start=(d == 0),
                            stop=(d == D_T - 1),
                        )
                    pu = pspool.tile([P, NTOK], f32, tag="pu", name="pu")
                    for d in range(D_T):
                        nc.tensor.matmul(
                            pu[:, :n],
                            lhsT=t3[:, d, c3:c3 + P],
                            rhs=xs[:, d, :],
                            start=(d == 0),
                            stop=(d == D_T - 1),
                        )
                    g = gpool.tile([P, NTOK], bf16, tag="g", name="g")
                    nc.scalar.activation(g[:, :n], pg[:, :n], Silu)
                    ht = hpool.tile([P, NTOK], bf16, tag=f"h_{h}", name=f"h_{h}")
                    nc.vector.tensor_mul(out=ht[:, :n], in0=g[:, :n], in1=pu[:, :n])
                    hts.append(ht)

                # stage 2: yT[do] = sum_h w2T[h,do].T @ hT[h]  -> [128, n] f32
                for do in range(D_T):
                    py = pspool.tile([P, NTOK], f32, tag="py", name="py", bufs=3)
                    for h in range(H_T):
                        nc.tensor.matmul(
                            py[:, :n],
                            lhsT=w2q[h // W2Q][:, h % W2Q, do * P:(do + 1) * P],
                            rhs=hts[h][:, :n],
                            start=(h == 0),
                            stop=(h == H_T - 1),
                        )
                    ot = opool.tile([P, NTOK], f32, tag="o", name="o")
                    nc.vector.tensor_copy(ot[:, :n], py[:, :n])
                    # output DMAs ride the ACT HWDGE ring, away from input
                    # loads; the last chunk's ride the by-then-idle SP ring
                    ring = nc.sync if ci == len(chunks) - 1 else nc.scalar
                    ring.dma_start(yT_d[do * P:(do + 1) * P, s0:s0 + n], ot[:, :n])

    nc.finalize()
    return nc


def kernel(x, router_w, w1, w2, w3):
    global LAST_RESULTS
    from concourse.bass_utils import run_bass_kernel_spmd

    x = np.ascontiguousarray(np.asarray(x, dtype=np.float32))
    router_w = np.asarray(router_w, dtype=np.float32)
    flat = x.reshape(T, D)

    # ---- host router (fp32, matches reference math) ----
    logits = flat @ router_w.T                      # [T, E]
    rows = np.arange(T)
    i1 = np.argmax(logits, axis=1)
    v1 = logits[rows, i1]
    masked = logits.copy()
    masked[rows, i1] = -np.inf
    i2 = np.argmax(masked, axis=1)
    v2 = masked[rows, i2]
    # softmax over the two selected logits (v1 >= v2)
    e2 = np.exp(v2 - v1)
    wt1 = 1.0 / (1.0 + e2)
    wt2 = e2 / (1.0 + e2)

    # ---- dispatch: token lists per expert ----
    idxs, wts = [], []
    for e in range(E):
        m1 = i1 == e
        m2 = i2 == e
        idx = np.nonzero(m1 | m2)[0]
        w = np.where(m1[idx], wt1[idx], wt2[idx]).astype(np.float32)
        idxs.append(idx)
        wts.append(w)
    max_cnt = max(len(i) for i in idxs)
    cap = max(NTOK, -(-max_cnt // 8) * 8)
    chunks = _chunk_sizes(cap)

    if cap not in _cache:
        _cache[cap] = _build_nc(cap)
    nc = _cache[cap]

    # ---- per-core inputs (bf16, pre-transposed, pre-packed) ----
    bf = ml_dtypes.bfloat16
    in_maps = []
    for e in range(E):
        idx = idxs[e]
        xTe = np.zeros((D, cap), dtype=bf)
        xTe[:, :len(idx)] = flat[idx].T.astype(bf)
        in_maps.append({
            "xT": _pack_x(xTe, chunks),
            "w1T": _pack_w13(np.ascontiguousarray(w1[e].T).astype(bf)),
            "w3T": _pack_w13(np.ascontiguousarray(w3[e].T).astype(bf)),
            "w2T": _pack_w2(np.ascontiguousarray(w2[e].T).astype(bf)),
        })

    trace = os.environ.get("KERNEL_TRACE", "0") == "1"
    kwargs = {}
    if trace:
        kwargs = dict(trace=True, trace_cores=list(range(E)))
    res = run_bass_kernel_spmd(nc, in_maps, core_ids=list(range(E)), **kwargs)
    LAST_RESULTS = res

    # ---- combine (the "all-to-all" return + weighted sum) ----
    out = np.zeros((T, D), dtype=np.float32)
    for e in range(E):
        idx = idxs[e]
        yT = res.results[e]["yT"]                   # [D, cap] f32
        out[idx] += wts[e][:, None] * yT[:, :len(idx)].T
    return out.reshape(B, S, D)
